# revision 35
# baseline (speedup 1.0000x reference)
"""CAPAttentionModule Trainium2 kernel (v3: fp8 DoubleRow + phase overlap).

Data-parallel over batch: 8 images -> 8 NeuronCores, one image per core.
Per core (x: [512, 9216] = [C, H*W], H=W=96):
  k1 = relu(Wkp x + b)   [128, HW]  fp8 DoubleRow conv (K=512, 2 instrs/tile)
  v1 = relu(Wvp x + b)   [256, HW]  fp8 DoubleRow conv
  q  = relu(Wq x + b)    [256, HW]  fp8 DoubleRow conv, kept fp8 for sim
  k2/v2 = relu(dw3x3+b)  diag matmuls: 4 fp8 DoubleRow tap-pairs + 1 single.
         Maps are stored 3x at byte offsets {0, +9615, +19230}; 9615 % 16 ==
         15 makes every cross-copy kt stride 16B-aligned, so ALL tap pairs
         satisfy the fp8 dual restrictions. Copies ride the gpsimd DMA ring.
  key/value = psp pooling  [*, 110]  strip reduces on DVE inside the dw window
  simT = key^T q / 16      [110, HW] ONE fp8 DoubleRow matmul per 512-chunk
  softmax over s (= partitions): exp on ACT; sum via ones-matmul broadcast;
         reciprocal_approx_fast + scale on DVE
  out = x + value @ simT   residual: half DVE adds, half identity-matmul
         PSUM accumulation + ACT copy
Phase B's sim/softmax pipeline is issued up to 5 chunks deep starting inside
the depthwise phase, so the PE never idles across the A/B boundary.
HBM traffic: read x fp8 (4.7MB) + x bf16 (9.4MB), write y bf16 (9.4MB).
"""

import numpy as np

P = 128
HH = 96
WP = 98          # padded width (zero border ring)
HW = 9216
HWP = WP * WP    # 9604
RB = 24          # row blocks of 4 rows
RBN = 4 * HH     # 384
NCH = 18         # phase-B column chunks
NCW = 512
S = 110
CSTR = 9615      # map copy stride (== 15 mod 16 -> aligned fp8 DR kt strides)
CSPAN = 1 + 2 * CSTR + HWP + 10   # one map chunk: lead pad + 3 copies


def build_bass():
    import concourse.bacc as bacc
    import concourse.tile as tile
    from concourse import mybir, bass
    from contextlib import ExitStack

    f32 = mybir.dt.float32
    f8 = mybir.dt.float8e4
    bf16 = mybir.dt.bfloat16
    AF = mybir.ActivationFunctionType
    AX = mybir.AxisListType
    ALU = mybir.AluOpType
    DR = mybir.MatmulPerfMode.DoubleRow

    nc = bacc.Bacc("TRN2", target_bir_lowering=False, debug=False,
                   enable_asserts=False, num_devices=8)

    x8_d = nc.dram_tensor("x8", [512, HW], f8, kind="ExternalInput").ap()
    xb_d = nc.dram_tensor("xb", [512, HW], bf16, kind="ExternalInput").ap()
    wq_d = nc.dram_tensor("wq", [512, 256], f8, kind="ExternalInput").ap()
    wkp_d = nc.dram_tensor("wkp", [512, 128], f8, kind="ExternalInput").ap()
    wvp_d = nc.dram_tensor("wvp", [512, 256], f8, kind="ExternalInput").ap()
    diag_d = nc.dram_tensor("diag", [3, 9, 128, 128], f8, kind="ExternalInput").ap()
    id_d = nc.dram_tensor("ident", [128, 128], bf16, kind="ExternalInput").ap()
    ones_d = nc.dram_tensor("ones", [128, 128], bf16, kind="ExternalInput").ap()
    scl_d = nc.dram_tensor("scl", [2, 128, S], f32, kind="ExternalInput").ap()
    bias_d = nc.dram_tensor("bias", [128, 8], f32, kind="ExternalInput").ap()
    y_d = nc.dram_tensor("y", [512, HW], bf16, kind="ExternalOutput").ap()

    x8_r = x8_d.rearrange("(t p) n -> p t n", p=P)
    xb_r = xb_d.rearrange("(t p) n -> p t n", p=P)
    y_r = y_d.rearrange("(t p) n -> p t n", p=P)

    with tile.TileContext(nc) as tc:
        with ExitStack() as top:
            cpool = top.enter_context(tc.tile_pool(name="consts", bufs=1))
            kpool = top.enter_context(tc.tile_pool(name="keep", bufs=1))
            # pools that span the A/B boundary
            pp = top.enter_context(tc.tile_pool(name="pexp", bufs=5))
            sp = top.enter_context(tc.tile_pool(name="small", bufs=3))
            xbp = top.enter_context(tc.tile_pool(name="xbt", bufs=4))

            c_wkp = cpool.tile([P, 4 * 128], f8)
            nc.scalar.dma_start(c_wkp[:].rearrange("p (t m) -> p t m", t=4),
                                wkp_d.rearrange("(t p) m -> p t m", p=P))
            c_wvp = cpool.tile([P, 4 * 256], f8)
            nc.scalar.dma_start(c_wvp[:].rearrange("p (t m) -> p t m", t=4),
                                wvp_d.rearrange("(t p) m -> p t m", p=P))
            c_wq = cpool.tile([P, 4 * 256], f8)
            nc.scalar.dma_start(c_wq[:].rearrange("p (t m) -> p t m", t=4),
                                wq_d.rearrange("(t p) m -> p t m", p=P))
            c_bias = cpool.tile([P, 8], f32)
            nc.scalar.dma_start(c_bias[:], bias_d)
            # deferred consts (first needed at dw / phase B) on scalar ring
            c_dg = cpool.tile([P, 27 * 128], f8)
            nc.scalar.dma_start(c_dg[:].rearrange("p (ct m) -> p ct m", ct=27),
                                diag_d.rearrange("c t p m -> p (c t) m"))
            c_id = cpool.tile([P, 128], bf16)
            nc.scalar.dma_start(c_id[:], id_d)
            c_ones = cpool.tile([P, 128], bf16)
            nc.scalar.dma_start(c_ones[:], ones_d)
            c_scl = cpool.tile([P, 2 * S], f32)
            nc.scalar.dma_start(c_scl[:].rearrange("p (s m) -> p s m", s=2),
                                scl_d.rearrange("s p m -> p s m"))

            keyn8 = kpool.tile([P, 2 * 128], f8)      # [kt, 110(+pad)] key/16
            vT = kpool.tile([S, 512], bf16)           # value^T [s, c]
            qsb8 = kpool.tile([P, 2 * HW], f8)        # q in fp8 [kq, n]

            # ---- phase-B helper state (pipeline spans into phase A) ----
            sims = [None] * NCH
            ets = [None] * NCH
            etns = [None] * NCH
            xbts = [None] * NCH
            bfn = {}

            def do_simT(n):
                knv = keyn8[:].rearrange("p (t m) -> p t m", t=2)[:, :, 0:S]
                qv = qsb8[:].rearrange("p (t m) -> p t m", t=2)
                ps = bfn["psS"].tile([S, NCW], f32, name="sim")
                nc.tensor.matmul(ps[:], knv, qv[:, :, n * NCW:(n + 1) * NCW],
                                 start=True, stop=True, perf_mode=DR)
                sims[n] = ps

            def do_exp(n):
                eT = pp.tile([S, NCW], bf16, name="eT")
                nc.scalar.activation(eT[:], sims[n][:], AF.Exp)
                ets[n] = eT

            def do_sumnorm(n):
                sb = bfn["psB"].tile([P, NCW], f32, name="sumbc")
                nc.tensor.matmul(sb[:], c_ones[0:S, :], ets[n][:],
                                 start=True, stop=True)
                rbc = sp.tile([S, NCW], f32, name="rbc")
                nc.vector.reciprocal_approx_fast(rbc[:], sb[0:S, :])
                eTn = pp.tile([S, NCW], bf16, name="eTn")
                nc.gpsimd.tensor_mul(eTn[:], ets[n][:], rbc[:])
                etns[n] = eTn

            def do_xbt(n):
                xbt = xbp.tile([P, 4 * NCW], bf16, name="xtb")
                nc.sync.dma_start(
                    xbt[:].rearrange("p (t n) -> p t n", t=4),
                    xb_r[:, :, n * NCW:(n + 1) * NCW])
                xbts[n] = xbt

            # ---------------- Phase A ----------------
            with ExitStack() as actx:
                bigp = actx.enter_context(tc.tile_pool(name="bigA", bufs=1))
                xap = actx.enter_context(tc.tile_pool(name="xa", bufs=3))
                blkp = actx.enter_context(tc.tile_pool(name="blk", bufs=3))
                tmpp = actx.enter_context(tc.tile_pool(name="tmpA", bufs=1))

                k1p = bigp.tile([P, CSPAN], f8)
                v1p = bigp.tile([P, 2 * CSPAN], f8)
                p24 = bigp.tile([P, 6 * 576], f32)
                allp = bigp.tile([P, 6 * S], f32)
                valn = bigp.tile([P, 4 * S], bf16)

                # zero pad borders (rows 0/97, cols 0/97 of each padded map)
                for chv in (k1p[:, 1:1 + HWP], v1p[:, 1:1 + HWP],
                            v1p[:, CSPAN + 1:CSPAN + 1 + HWP]):
                    c3 = chv.rearrange("p (h w) -> p h w", w=WP)
                    nc.gpsimd.memset(c3[:, 0:1, :], 0.0)
                    nc.gpsimd.memset(c3[:, 97:98, :], 0.0)
                    nc.gpsimd.memset(c3[:, 1:97, 0:1], 0.0)
                    nc.gpsimd.memset(c3[:, 1:97, 97:98], 0.0)

                def strip_pool(mapap, base, slot, strip):
                    # 4x4 block sums of 24 rows (6 rowblocks) -> p24
                    off = base + (24 * strip + 1) * WP + 1
                    src = bass.AP(tensor=mapap.tensor, offset=mapap.offset + off,
                                  ap=[mapap.ap[0], [4 * WP, 6], [4, 24],
                                      [WP, 4], [1, 4]])
                    dst = p24[:, slot * 576 + strip * 144:slot * 576 + (strip + 1) * 144]
                    nc.vector.tensor_reduce(
                        dst.rearrange("p (a b) -> p a b", a=6), src,
                        axis=AX.XY, op=ALU.add)

                def strip_copy(mapap, cb, strip):
                    # replicate newly-written strip rows into copies 2 and 3
                    r0 = [0, 2450, 4802, 7154][strip]
                    r1 = [2450, 4802, 7154, 9604][strip]
                    src = mapap[:, cb + 1 + r0:cb + 1 + r1]
                    for cc in (1, 2):
                        nc.gpsimd.dma_start(
                            mapap[:, cb + 1 + cc * CSTR + r0:cb + 1 + cc * CSTR + r1],
                            src)

                # primary 1x1 convs + q conv, fp8 DoubleRow, 2 rowblocks/DMA
                with tc.tile_pool(name="psA", bufs=2, space="PSUM") as psA, \
                        tc.tile_pool(name="psQ", bufs=1, space="PSUM") as psQ:
                    for rbb in range(RB // 2):
                        xt = xap.tile([P, 4 * 2 * RBN], f8, name="xt")
                        nc.sync.dma_start(
                            xt[:].rearrange("p (t n) -> p t n", t=4),
                            x8_r[:, :, rbb * 2 * RBN:(rbb + 1) * 2 * RBN])
                        xtv = xt[:].rearrange("p (t n) -> p t n", t=4)
                        wqv = c_wq[:].rearrange("p (t m) -> p t m", t=4)
                        wkv = c_wkp[:].rearrange("p (t m) -> p t m", t=4)
                        wvv = c_wvp[:].rearrange("p (t m) -> p t m", t=4)
                        for sub in range(2):
                            rb = rbb * 2 + sub
                            rhs0 = xtv[:, 0:2, sub * RBN:(sub + 1) * RBN]
                            rhs1 = xtv[:, 2:4, sub * RBN:(sub + 1) * RBN]
                            dsts = [(k1p, 0, wkv[:, :, 0:128], 0),
                                    (v1p, 0, wvv[:, :, 0:128], 2),
                                    (v1p, 1, wvv[:, :, 128:256], 3)]
                            for di, (dst, half, wt, bcol) in enumerate(dsts):
                                ps = psA.tile([P, RBN], f32, name=f"pps{di}")
                                nc.tensor.matmul(ps[:], wt[:, 0:2, :], rhs0,
                                                 start=True, stop=False, perf_mode=DR)
                                nc.tensor.matmul(ps[:], wt[:, 2:4, :], rhs1,
                                                 start=False, stop=True, perf_mode=DR)
                                dv = dst[:, half * CSPAN + 1:half * CSPAN + 1 + HWP].rearrange(
                                    "p (h w) -> p h w", w=WP)
                                dvv = dv[:, 4 * rb + 1:4 * rb + 5, 1:97]
                                psv = ps[:].rearrange("p (h w) -> p h w", w=HH)
                                if di == 2 and rb % 2 == 0:
                                    nc.vector.tensor_scalar(
                                        dvv, psv, c_bias[:, bcol:bcol + 1],
                                        0.0, ALU.add, ALU.max)
                                else:
                                    nc.scalar.activation(
                                        dvv, psv, AF.Relu,
                                        bias=c_bias[:, bcol:bcol + 1])
                            for kq in range(2):
                                qps = psQ.tile([P, RBN], f32, name=f"q{kq}")
                                nc.tensor.matmul(qps[:], wqv[:, 0:2, kq * 128:kq * 128 + 128],
                                                 rhs0, start=True, stop=False, perf_mode=DR)
                                nc.tensor.matmul(qps[:], wqv[:, 2:4, kq * 128:kq * 128 + 128],
                                                 rhs1, start=False, stop=True, perf_mode=DR)
                                qdst = qsb8[:, kq * HW + rb * RBN:kq * HW + (rb + 1) * RBN]
                                if kq == 0:
                                    nc.vector.tensor_scalar(
                                        qdst, qps[:], c_bias[:, 6:7], 0.0,
                                        ALU.add, ALU.max)
                                else:
                                    nc.scalar.activation(
                                        qdst, qps[:], AF.Relu,
                                        bias=c_bias[:, 7:8])
                        if rbb % 3 == 2:
                            strip = rbb // 3
                            strip_copy(k1p[:], 0, strip)
                            strip_copy(v1p[:], 0, strip)
                            strip_copy(v1p[:], CSPAN, strip)
                            strip_pool(k1p[:], 1, 0, strip)
                            strip_pool(v1p[:], 1, 2, strip)
                            strip_pool(v1p[:], CSPAN + 1, 3, strip)

                # phase-B psum pools: allocated now (primary pools released)
                # on the top stack so they survive into the ctx loop
                bfn["psS"] = top.enter_context(
                    tc.tile_pool(name="psS", bufs=2, space="PSUM"))
                bfn["psB"] = top.enter_context(
                    tc.tile_pool(name="psB", bufs=2, space="PSUM"))

                # small pools over map range [m0, m1) -> allp columns
                def smallpools(m0, m1):
                    m = m1 - m0
                    allp_v = allp[:, m0 * S:m1 * S].rearrange(
                        "p (m s) -> p m s", s=S)
                    p24s = p24[:, m0 * 576:m1 * 576]
                    nc.vector.reduce_sum(
                        allp_v[:, :, 0:1],
                        p24s.rearrange("p (m s) -> p m s", s=576), axis=AX.X)
                    tmp = tmpp.tile([P, 1152], f32, name="tmp", tag="tmp")
                    nc.vector.reduce_sum(
                        tmp[:, 0:m * 72],
                        p24s.rearrange("p (mh wq ws) -> p mh wq ws", wq=3, ws=8),
                        axis=AX.X)
                    nc.vector.reduce_sum(
                        allp_v[:, :, 1:10],
                        tmp[:, 0:m * 72].rearrange(
                            "p (m hq hs wq) -> p m hq wq hs", m=m, hq=3, hs=8),
                        axis=AX.X)
                    tmp6 = tmpp.tile([P, 1152], f32, name="tmp6", tag="tmp")
                    nc.vector.reduce_sum(
                        tmp6[:, 0:m * 144],
                        p24s.rearrange("p (mh wq ws) -> p mh wq ws", wq=6, ws=4),
                        axis=AX.X)
                    nc.vector.reduce_sum(
                        allp_v[:, :, 10:46],
                        tmp6[:, 0:m * 144].rearrange(
                            "p (m hq hs wq) -> p m hq wq hs", m=m, hq=6, hs=4),
                        axis=AX.X)
                    tmp8 = tmpp.tile([P, 1152], f32, name="tmp8", tag="tmp")
                    nc.vector.reduce_sum(
                        tmp8[:, 0:m * 192],
                        p24s.rearrange("p (mh wq ws) -> p mh wq ws", wq=8, ws=3),
                        axis=AX.X)
                    nc.vector.reduce_sum(
                        allp_v[:, :, 46:110],
                        tmp8[:, 0:m * 192].rearrange(
                            "p (m hq hs wq) -> p m hq wq hs", m=m, hq=8, hs=3),
                        axis=AX.X)

                # depthwise 3x3: 4 fp8-DR tap pairs + 1 single per rowblock
                with tc.tile_pool(name="psD", bufs=1, space="PSUM") as psD, \
                        tc.tile_pool(name="psTa", bufs=1, space="PSUM") as psTp:

                    def vt_build(j):
                        tp = psTp.tile([P, 128], bf16, name="tp", tag="tp")
                        nc.tensor.transpose(tp[0:S, :], valn[:, j * S:(j + 1) * S],
                                            c_id[:])
                        nc.scalar.copy(vT[:, j * 128:(j + 1) * 128], tp[0:S, :])

                    def val_pools(m0, m1):
                        smallpools(m0, m1)
                        for mm in range(m0, m1):
                            j = mm - 2
                            nc.gpsimd.tensor_mul(valn[:, j * S:(j + 1) * S],
                                                 allp[:, mm * S:(mm + 1) * S],
                                                 c_scl[:, S:2 * S])

                    def val_finish(m0, m1):
                        val_pools(m0, m1)
                        for mm in range(m0, m1):
                            vt_build(mm - 2)

                    dgv = c_dg[:].rearrange("p (ct m) -> p ct m", ct=27)
                    # tap pairs for fp8 DoubleRow: (ta, kt0 copy idx, kt delta)
                    pairs = [(0, 0, CSTR + 1), (2, 0, 96),
                             (5, 0, 96), (7, 1, CSTR + 1)]
                    pend = []
                    for ci, bcol, slot in ((0, 1, 1), (1, 4, 4), (2, 5, 5)):
                        chv = (v1p[:, CSPAN:2 * CSPAN] if ci == 2 else
                               (v1p[:, 0:CSPAN] if ci == 1 else k1p[:, 0:CSPAN]))
                        m3 = chv[:, 1:1 + HWP].rearrange("p (h w) -> p h w", w=WP)
                        for g in range(8):
                            blk = blkp.tile([P, 3 * RBN], bf16, name="blk")
                            pss = [psD.tile([P, 392], f32, name=f"dw{j}")
                                   for j in range(3)]
                            for pi, (ta, cpi, delta) in enumerate(pairs):
                                dya, dxa = ta // 3, ta % 3
                                lhsT = dgv[:, ci * 9 + ta:ci * 9 + ta + 2, :]
                                for j in range(3):
                                    rb = g * 3 + j
                                    off = (cpi * CSTR + 1
                                           + (4 * rb + dya) * WP + (dxa - 1))
                                    va = chv[:, off:off + 392]
                                    rhs = bass.AP(
                                        tensor=va.tensor, offset=va.offset,
                                        ap=[va.ap[0], [delta, 2], [1, 392]])
                                    nc.tensor.matmul(
                                        pss[j][:], lhsT, rhs,
                                        start=(pi == 0), stop=False, perf_mode=DR)
                            for j in range(3):
                                rb = g * 3 + j
                                dwv = pss[j][:].rearrange(
                                    "p (h w) -> p h w", w=WP)[:, :, 1:97]
                                nc.tensor.matmul(
                                    dwv, dgv[:, ci * 9 + 4, :],
                                    m3[:, 4 * rb + 1:4 * rb + 5, 1:97],
                                    start=False, stop=True)
                                nc.scalar.activation(
                                    blk[:, j * RBN:(j + 1) * RBN].rearrange(
                                        "p (h w) -> p h w", w=96),
                                    dwv, AF.Relu, bias=c_bias[:, bcol:bcol + 1])
                            bv = blk[:].rearrange(
                                "p (r h wq ws) -> p r wq h ws", r=3, h=4, ws=4)
                            nc.vector.tensor_reduce(
                                p24[:, slot * 576 + g * 72:slot * 576 + (g + 1) * 72]
                                .rearrange("p (a b) -> p a b", a=3),
                                bv, axis=AX.XY, op=ALU.add)
                            # spread previous chunk's finish work across groups
                            if pend and g % 2 == 1:
                                pend.pop(0)()
                        while pend:
                            pend.pop(0)()

                        def key_done():
                            smallpools(0, 2)
                            for kq in range(2):
                                nc.vector.tensor_mul(
                                    keyn8[:, kq * 128:kq * 128 + S],
                                    allp[:, kq * S:(kq + 1) * S], c_scl[:, 0:S])
                            do_simT(0); do_exp(0)

                        def b_pre0():
                            do_sumnorm(0); do_simT(1); do_exp(1)
                            do_xbt(0); do_xbt(1)

                        def b_pre1():
                            do_sumnorm(1); do_simT(2); do_exp(2)

                        if ci == 0:
                            pend = [key_done, b_pre0,
                                    lambda: val_pools(2, 4),
                                    lambda: (vt_build(0), vt_build(1))]
                        elif ci == 1:
                            pend = [lambda: val_pools(4, 5), b_pre1,
                                    lambda: vt_build(2)]
                        else:
                            val_pools(5, 6)
                            do_sumnorm(2); do_simT(3); do_exp(3)
                            do_simT(4); do_exp(4)
                            vt_build(3)

            # ---------------- Phase B: context / output ----------------
            with ExitStack() as bctx:
                obp = bctx.enter_context(tc.tile_pool(name="outb", bufs=3))
                psC = bctx.enter_context(tc.tile_pool(name="psC", bufs=4, space="PSUM"))

                for n in range(NCH):
                    if n + 5 < NCH:
                        do_simT(n + 5)
                        do_exp(n + 5)
                    if n + 3 < NCH:
                        do_sumnorm(n + 3)
                    if n + 2 < NCH:
                        do_xbt(n + 2)
                    outb = obp.tile([P, 4 * NCW], bf16, name="outb")
                    xbt = xbts[n][:]
                    for cv in range(4):
                        cps = psC.tile([P, NCW], f32, name="ctx")
                        nc.tensor.matmul(cps[:], vT[:, cv * 128:(cv + 1) * 128],
                                         etns[n][:], start=True,
                                         stop=(cv < 2))
                        if cv < 2:
                            nc.vector.tensor_add(
                                outb[:, cv * NCW:(cv + 1) * NCW], cps[:],
                                xbt[:, cv * NCW:(cv + 1) * NCW])
                        else:
                            nc.tensor.matmul(cps[:], c_id[:],
                                             xbt[:, cv * NCW:(cv + 1) * NCW],
                                             start=False, stop=True)
                            nc.scalar.copy(outb[:, cv * NCW:(cv + 1) * NCW],
                                           cps[:])
                    nc.scalar.dma_start(
                        y_r[:, :, n * NCW:(n + 1) * NCW],
                        outb[:].rearrange("p (t n) -> p t n", t=4))

    nc.compile()
    return nc


def prep_host_inputs(inputs):
    """Fold BN affine into weights, quantize to fp8, build aux tensors."""
    import ml_dtypes
    F8 = ml_dtypes.float8_e4m3
    g = lambda a: np.ascontiguousarray(np.asarray(a, dtype=np.float32))
    wq = (g(inputs["q_g"])[:, None] * g(inputs["q_w"])[:, :, 0, 0]).T
    wkp = (g(inputs["kp_g"])[:, None] * g(inputs["kp_w"])[:, :, 0, 0]).T
    wvp = (g(inputs["vp_g"])[:, None] * g(inputs["vp_w"])[:, :, 0, 0]).T
    wkc = g(inputs["kc_g"])[:, None] * g(inputs["kc_w"])[:, 0].reshape(128, 9)
    wvc = g(inputs["vc_g"])[:, None] * g(inputs["vc_w"])[:, 0].reshape(256, 9)

    diag = np.zeros((3, 9, 128, 128), np.float32)
    for t in range(9):
        diag[0, t] = np.diag(wkc[:, t])
        diag[1, t] = np.diag(wvc[:128, t])
        diag[2, t] = np.diag(wvc[128:, t])

    scale110 = np.zeros(S, np.float32)
    scale110[0] = 1.0 / 9216
    scale110[1:10] = 1.0 / 1024
    scale110[10:46] = 1.0 / 256
    scale110[46:110] = 1.0 / 144
    scl = np.zeros((2, 128, S), np.float32)
    scl[0] = scale110 / 16.0
    scl[1] = scale110

    bias = np.zeros((128, 8), np.float32)
    bias[:, 0] = g(inputs["kp_b"])
    bias[:, 1] = g(inputs["kc_b"])
    bias[:, 2] = g(inputs["vp_b"])[:128]
    bias[:, 3] = g(inputs["vp_b"])[128:]
    bias[:, 4] = g(inputs["vc_b"])[:128]
    bias[:, 5] = g(inputs["vc_b"])[128:]
    bias[:, 6] = g(inputs["q_b"])[:128]
    bias[:, 7] = g(inputs["q_b"])[128:]

    return {
        "wq": np.ascontiguousarray(wq).astype(F8),
        "wkp": np.ascontiguousarray(wkp).astype(F8),
        "wvp": np.ascontiguousarray(wvp).astype(F8),
        "diag": diag.astype(F8),
        "ident": np.eye(128, dtype=ml_dtypes.bfloat16),
        "ones": np.ones((128, 128), dtype=ml_dtypes.bfloat16),
        "scl": scl,
        "bias": bias,
    }


def make_in_maps(inputs):
    import ml_dtypes
    host = prep_host_inputs(inputs)
    x = np.asarray(inputs["x"], dtype=np.float32)
    B = x.shape[0]
    in_maps = []
    for b in range(B):
        m = dict(host)
        xf = np.ascontiguousarray(x[b].reshape(512, HW))
        m["x8"] = xf.astype(ml_dtypes.float8_e4m3)
        m["xb"] = xf.astype(ml_dtypes.bfloat16)
        in_maps.append(m)
    return in_maps


_NC = None


def get_nc():
    global _NC
    if _NC is None:
        _NC = build_bass()
    return _NC


def kernel(**inputs):
    from concourse import bass_utils
    nc = get_nc()
    in_maps = make_in_maps(inputs)
    res = bass_utils.run_bass_kernel_spmd(
        nc, in_maps, core_ids=list(range(len(in_maps))), trace=False)
    outs = [r["y"].astype(np.float32).reshape(512, HH, HH) for r in res.results]
    return np.stack(outs, axis=0).astype(np.float32)


# revision 36
# speedup vs baseline: 1.1565x; 1.1565x over previous
"""CAPAttentionModule Trainium2 kernel (v3: fp8 DoubleRow + phase overlap).

Data-parallel over batch: 8 images -> 8 NeuronCores, one image per core.
Per core (x: [512, 9216] = [C, H*W], H=W=96):
  k1 = relu(Wkp x + b)   [128, HW]  fp8 DoubleRow conv (K=512, 2 instrs/tile)
  v1 = relu(Wvp x + b)   [256, HW]  fp8 DoubleRow conv
  q  = relu(Wq x + b)    [256, HW]  fp8 DoubleRow conv, kept fp8 for sim
  k2/v2 = relu(dw3x3+b)  diag matmuls: 4 fp8 DoubleRow tap-pairs + 1 single.
         Maps are stored 3x at byte offsets {0, +9615, +19230}; 9615 % 16 ==
         15 makes every cross-copy kt stride 16B-aligned, so ALL tap pairs
         satisfy the fp8 dual restrictions. Copies ride the gpsimd DMA ring.
  key/value = psp pooling  [*, 110]  strip reduces on DVE inside the dw window
  simT = key^T q / 16      [110, HW] ONE fp8 DoubleRow matmul per 512-chunk
  softmax over s (= partitions): exp on ACT; sum via ones-matmul broadcast;
         reciprocal_approx_fast + scale on DVE
  out = x + value @ simT   residual: half DVE adds, half identity-matmul
         PSUM accumulation + ACT copy
Phase B's sim/softmax pipeline is issued up to 5 chunks deep starting inside
the depthwise phase, so the PE never idles across the A/B boundary.
HBM traffic: read x fp8 (4.7MB) + x bf16 (9.4MB), write y bf16 (9.4MB).
"""

import numpy as np

P = 128
HH = 96
WP = 98          # padded width (zero border ring)
HW = 9216
HWP = WP * WP    # 9604
RB = 24          # row blocks of 4 rows
RBN = 4 * HH     # 384
NCH = 18         # phase-B column chunks
NCW = 512
S = 110
CSTR = 9615      # map copy stride (== 15 mod 16 -> aligned fp8 DR kt strides)
CSPAN = 1 + 2 * CSTR + HWP + 10   # one map chunk: lead pad + 3 copies


def build_bass():
    import concourse.bacc as bacc
    import concourse.tile as tile
    from concourse import mybir, bass
    from contextlib import ExitStack

    f32 = mybir.dt.float32
    f8 = mybir.dt.float8e4
    bf16 = mybir.dt.bfloat16
    AF = mybir.ActivationFunctionType
    AX = mybir.AxisListType
    ALU = mybir.AluOpType
    DR = mybir.MatmulPerfMode.DoubleRow

    nc = bacc.Bacc("TRN2", target_bir_lowering=False, debug=False,
                   enable_asserts=False, num_devices=8)

    x8_d = nc.dram_tensor("x8", [512, HW], f8, kind="ExternalInput").ap()
    xb_d = nc.dram_tensor("xb", [512, HW], bf16, kind="ExternalInput").ap()
    wq_d = nc.dram_tensor("wq", [512, 256], f8, kind="ExternalInput").ap()
    wkp_d = nc.dram_tensor("wkp", [512, 128], f8, kind="ExternalInput").ap()
    wvp_d = nc.dram_tensor("wvp", [512, 256], f8, kind="ExternalInput").ap()
    diag_d = nc.dram_tensor("diag", [3, 9, 128, 128], f8, kind="ExternalInput").ap()
    id_d = nc.dram_tensor("ident", [128, 128], bf16, kind="ExternalInput").ap()
    ones_d = nc.dram_tensor("ones", [128, 128], bf16, kind="ExternalInput").ap()
    scl_d = nc.dram_tensor("scl", [2, 128, S], f32, kind="ExternalInput").ap()
    bias_d = nc.dram_tensor("bias", [128, 8], f32, kind="ExternalInput").ap()
    y_d = nc.dram_tensor("y", [512, HW], bf16, kind="ExternalOutput").ap()

    x8_r = x8_d.rearrange("(t p) n -> p t n", p=P)
    xb_r = xb_d.rearrange("(t p) n -> p t n", p=P)
    y_r = y_d.rearrange("(t p) n -> p t n", p=P)

    with tile.TileContext(nc) as tc:
        with ExitStack() as top:
            cpool = top.enter_context(tc.tile_pool(name="consts", bufs=1))
            kpool = top.enter_context(tc.tile_pool(name="keep", bufs=1))
            # pools that span the A/B boundary
            pp = top.enter_context(tc.tile_pool(name="pexp", bufs=5))
            sp = top.enter_context(tc.tile_pool(name="small", bufs=3))
            xbp = top.enter_context(tc.tile_pool(name="xbt", bufs=4))

            c_wkp = cpool.tile([P, 4 * 128], f8)
            nc.scalar.dma_start(c_wkp[:].rearrange("p (t m) -> p t m", t=4),
                                wkp_d.rearrange("(t p) m -> p t m", p=P))
            c_wvp = cpool.tile([P, 4 * 256], f8)
            nc.scalar.dma_start(c_wvp[:].rearrange("p (t m) -> p t m", t=4),
                                wvp_d.rearrange("(t p) m -> p t m", p=P))
            c_wq = cpool.tile([P, 4 * 256], f8)
            nc.scalar.dma_start(c_wq[:].rearrange("p (t m) -> p t m", t=4),
                                wq_d.rearrange("(t p) m -> p t m", p=P))
            c_bias = cpool.tile([P, 8], f32)
            nc.scalar.dma_start(c_bias[:], bias_d)
            # deferred consts (first needed at dw / phase B) on scalar ring
            c_dg = cpool.tile([P, 27 * 128], f8)
            nc.scalar.dma_start(c_dg[:].rearrange("p (ct m) -> p ct m", ct=27),
                                diag_d.rearrange("c t p m -> p (c t) m"))
            c_id = cpool.tile([P, 128], bf16)
            nc.scalar.dma_start(c_id[:], id_d)
            c_ones = cpool.tile([P, 128], bf16)
            nc.scalar.dma_start(c_ones[:], ones_d)
            c_scl = cpool.tile([P, 2 * S], f32)
            nc.scalar.dma_start(c_scl[:].rearrange("p (s m) -> p s m", s=2),
                                scl_d.rearrange("s p m -> p s m"))

            keyn8 = kpool.tile([P, 2 * 128], f8)      # [kt, 110(+pad)] key/16
            vT = kpool.tile([S, 512], bf16)           # value^T [s, c]
            qsb8 = kpool.tile([P, 2 * HW], f8)        # q in fp8 [kq, n]

            # ---- phase-B helper state (pipeline spans into phase A) ----
            sims = [None] * NCH
            ets = [None] * NCH
            etns = [None] * NCH
            xbts = [None] * NCH
            bfn = {}

            def do_simT(n):
                knv = keyn8[:].rearrange("p (t m) -> p t m", t=2)[:, :, 0:S]
                qv = qsb8[:].rearrange("p (t m) -> p t m", t=2)
                ps = bfn["psS"].tile([S, NCW], f32, name="sim")
                nc.tensor.matmul(ps[:], knv, qv[:, :, n * NCW:(n + 1) * NCW],
                                 start=True, stop=True, perf_mode=DR)
                sims[n] = ps

            def do_exp(n):
                eT = pp.tile([S, NCW], bf16, name="eT")
                nc.scalar.activation(eT[:], sims[n][:], AF.Exp)
                ets[n] = eT

            def do_sumnorm(n):
                sb = bfn["psB"].tile([P, NCW], f32, name="sumbc")
                nc.tensor.matmul(sb[:], c_ones[0:S, :], ets[n][:],
                                 start=True, stop=True)
                rbc = sp.tile([S, NCW], f32, name="rbc")
                nc.vector.reciprocal_approx_fast(rbc[:], sb[0:S, :])
                eTn = pp.tile([S, NCW], bf16, name="eTn")
                nc.gpsimd.tensor_mul(eTn[:], ets[n][:], rbc[:])
                etns[n] = eTn

            def do_xbt(n):
                xbt = xbp.tile([P, 4 * NCW], bf16, name="xtb")
                nc.sync.dma_start(
                    xbt[:].rearrange("p (t n) -> p t n", t=4),
                    xb_r[:, :, n * NCW:(n + 1) * NCW])
                xbts[n] = xbt

            # ---------------- Phase A ----------------
            with ExitStack() as actx:
                bigp = actx.enter_context(tc.tile_pool(name="bigA", bufs=1))
                xap = actx.enter_context(tc.tile_pool(name="xa", bufs=3))
                blkp = actx.enter_context(tc.tile_pool(name="blk", bufs=3))
                tmpp = actx.enter_context(tc.tile_pool(name="tmpA", bufs=1))

                k1p = bigp.tile([P, CSPAN], f8)
                v1p = bigp.tile([P, 2 * CSPAN], f8)
                p24 = bigp.tile([P, 6 * 576], f32)
                allp = bigp.tile([P, 6 * S], f32)
                valn = bigp.tile([P, 4 * S], bf16)

                # zero pad borders (rows 0/97, cols 0/97 of each padded map)
                for chv in (k1p[:, 1:1 + HWP], v1p[:, 1:1 + HWP],
                            v1p[:, CSPAN + 1:CSPAN + 1 + HWP]):
                    c3 = chv.rearrange("p (h w) -> p h w", w=WP)
                    nc.gpsimd.memset(c3[:, 0:1, :], 0.0)
                    nc.gpsimd.memset(c3[:, 97:98, :], 0.0)
                    nc.gpsimd.memset(c3[:, 1:97, 0:1], 0.0)
                    nc.gpsimd.memset(c3[:, 1:97, 97:98], 0.0)

                def strip_pool(mapap, base, slot, strip):
                    # 4x4 block sums of 24 rows (6 rowblocks) -> p24
                    off = base + (24 * strip + 1) * WP + 1
                    src = bass.AP(tensor=mapap.tensor, offset=mapap.offset + off,
                                  ap=[mapap.ap[0], [4 * WP, 6], [4, 24],
                                      [WP, 4], [1, 4]])
                    dst = p24[:, slot * 576 + strip * 144:slot * 576 + (strip + 1) * 144]
                    nc.vector.tensor_reduce(
                        dst.rearrange("p (a b) -> p a b", a=6), src,
                        axis=AX.XY, op=ALU.add)

                def strip_copy(mapap, cb, strip):
                    # replicate newly-written strip rows into copies 2 and 3
                    r0 = [0, 2450, 4802, 7154][strip]
                    r1 = [2450, 4802, 7154, 9604][strip]
                    src = mapap[:, cb + 1 + r0:cb + 1 + r1]
                    for cc in (1, 2):
                        nc.gpsimd.dma_start(
                            mapap[:, cb + 1 + cc * CSTR + r0:cb + 1 + cc * CSTR + r1],
                            src)

                # primary 1x1 convs + q conv, fp8 DoubleRow, 2 rowblocks/DMA
                with tc.tile_pool(name="psA", bufs=2, space="PSUM") as psA, \
                        tc.tile_pool(name="psQ", bufs=1, space="PSUM") as psQ:
                    for rbb in range(RB // 2):
                        xt = xap.tile([P, 4 * 2 * RBN], f8, name="xt")
                        nc.sync.dma_start(
                            xt[:].rearrange("p (t n) -> p t n", t=4),
                            x8_r[:, :, rbb * 2 * RBN:(rbb + 1) * 2 * RBN])
                        xtv = xt[:].rearrange("p (t n) -> p t n", t=4)
                        wqv = c_wq[:].rearrange("p (t m) -> p t m", t=4)
                        wkv = c_wkp[:].rearrange("p (t m) -> p t m", t=4)
                        wvv = c_wvp[:].rearrange("p (t m) -> p t m", t=4)
                        for sub in range(2):
                            rb = rbb * 2 + sub
                            rhs0 = xtv[:, 0:2, sub * RBN:(sub + 1) * RBN]
                            rhs1 = xtv[:, 2:4, sub * RBN:(sub + 1) * RBN]
                            dsts = [(k1p, 0, wkv[:, :, 0:128], 0),
                                    (v1p, 0, wvv[:, :, 0:128], 2),
                                    (v1p, 1, wvv[:, :, 128:256], 3)]
                            for di, (dst, half, wt, bcol) in enumerate(dsts):
                                ps = psA.tile([P, RBN], f32, name=f"pps{di}")
                                nc.tensor.matmul(ps[:], wt[:, 0:2, :], rhs0,
                                                 start=True, stop=False, perf_mode=DR)
                                nc.tensor.matmul(ps[:], wt[:, 2:4, :], rhs1,
                                                 start=False, stop=True, perf_mode=DR)
                                dv = dst[:, half * CSPAN + 1:half * CSPAN + 1 + HWP].rearrange(
                                    "p (h w) -> p h w", w=WP)
                                dvv = dv[:, 4 * rb + 1:4 * rb + 5, 1:97]
                                psv = ps[:].rearrange("p (h w) -> p h w", w=HH)
                                if di == 2 and rb % 2 == 0:
                                    nc.vector.tensor_scalar(
                                        dvv, psv, c_bias[:, bcol:bcol + 1],
                                        0.0, ALU.add, ALU.max)
                                else:
                                    nc.scalar.activation(
                                        dvv, psv, AF.Relu,
                                        bias=c_bias[:, bcol:bcol + 1])
                            for kq in range(2):
                                qps = psQ.tile([P, RBN], f32, name=f"q{kq}")
                                nc.tensor.matmul(qps[:], wqv[:, 0:2, kq * 128:kq * 128 + 128],
                                                 rhs0, start=True, stop=False, perf_mode=DR)
                                nc.tensor.matmul(qps[:], wqv[:, 2:4, kq * 128:kq * 128 + 128],
                                                 rhs1, start=False, stop=True, perf_mode=DR)
                                qdst = qsb8[:, kq * HW + rb * RBN:kq * HW + (rb + 1) * RBN]
                                if kq == 0:
                                    nc.vector.tensor_scalar(
                                        qdst, qps[:], c_bias[:, 6:7], 0.0,
                                        ALU.add, ALU.max)
                                else:
                                    nc.scalar.activation(
                                        qdst, qps[:], AF.Relu,
                                        bias=c_bias[:, 7:8])
                        if rbb % 3 == 2:
                            strip = rbb // 3
                            strip_copy(k1p[:], 0, strip)
                            strip_copy(v1p[:], 0, strip)
                            strip_copy(v1p[:], CSPAN, strip)
                            strip_pool(k1p[:], 1, 0, strip)
                            strip_pool(v1p[:], 1, 2, strip)
                            strip_pool(v1p[:], CSPAN + 1, 3, strip)

                # phase-B psum pools: allocated now (primary pools released)
                # on the top stack so they survive into the ctx loop
                bfn["psS"] = top.enter_context(
                    tc.tile_pool(name="psS", bufs=2, space="PSUM"))
                bfn["psB"] = top.enter_context(
                    tc.tile_pool(name="psB", bufs=1, space="PSUM"))

                # small pools over map range [m0, m1) -> allp columns
                def smallpools(m0, m1):
                    m = m1 - m0
                    allp_v = allp[:, m0 * S:m1 * S].rearrange(
                        "p (m s) -> p m s", s=S)
                    p24s = p24[:, m0 * 576:m1 * 576]
                    nc.vector.reduce_sum(
                        allp_v[:, :, 0:1],
                        p24s.rearrange("p (m s) -> p m s", s=576), axis=AX.X)
                    tmp = tmpp.tile([P, 1152], f32, name="tmp", tag="tmp")
                    nc.vector.reduce_sum(
                        tmp[:, 0:m * 72],
                        p24s.rearrange("p (mh wq ws) -> p mh wq ws", wq=3, ws=8),
                        axis=AX.X)
                    nc.vector.reduce_sum(
                        allp_v[:, :, 1:10],
                        tmp[:, 0:m * 72].rearrange(
                            "p (m hq hs wq) -> p m hq wq hs", m=m, hq=3, hs=8),
                        axis=AX.X)
                    tmp6 = tmpp.tile([P, 1152], f32, name="tmp6", tag="tmp")
                    nc.vector.reduce_sum(
                        tmp6[:, 0:m * 144],
                        p24s.rearrange("p (mh wq ws) -> p mh wq ws", wq=6, ws=4),
                        axis=AX.X)
                    nc.vector.reduce_sum(
                        allp_v[:, :, 10:46],
                        tmp6[:, 0:m * 144].rearrange(
                            "p (m hq hs wq) -> p m hq wq hs", m=m, hq=6, hs=4),
                        axis=AX.X)
                    tmp8 = tmpp.tile([P, 1152], f32, name="tmp8", tag="tmp")
                    nc.vector.reduce_sum(
                        tmp8[:, 0:m * 192],
                        p24s.rearrange("p (mh wq ws) -> p mh wq ws", wq=8, ws=3),
                        axis=AX.X)
                    nc.vector.reduce_sum(
                        allp_v[:, :, 46:110],
                        tmp8[:, 0:m * 192].rearrange(
                            "p (m hq hs wq) -> p m hq wq hs", m=m, hq=8, hs=3),
                        axis=AX.X)

                # depthwise 3x3: 4 fp8-DR tap pairs + 1 single per rowblock
                with tc.tile_pool(name="psD", bufs=1, space="PSUM") as psD, \
                        tc.tile_pool(name="psTa", bufs=2, space="PSUM") as psTp:

                    def vt_build(j):
                        tp = psTp.tile([P, 128], bf16, name="tp", tag="tp")
                        nc.tensor.transpose(tp[0:S, :], valn[:, j * S:(j + 1) * S],
                                            c_id[:])
                        nc.scalar.copy(vT[:, j * 128:(j + 1) * 128], tp[0:S, :])

                    def val_pools(m0, m1):
                        smallpools(m0, m1)
                        for mm in range(m0, m1):
                            j = mm - 2
                            nc.gpsimd.tensor_mul(valn[:, j * S:(j + 1) * S],
                                                 allp[:, mm * S:(mm + 1) * S],
                                                 c_scl[:, S:2 * S])

                    def val_finish(m0, m1):
                        val_pools(m0, m1)
                        for mm in range(m0, m1):
                            vt_build(mm - 2)

                    dgv = c_dg[:].rearrange("p (ct m) -> p ct m", ct=27)
                    # tap pairs for fp8 DoubleRow: (ta, kt0 copy idx, kt delta)
                    pairs = [(0, 0, CSTR + 1), (2, 0, 96),
                             (5, 0, 96), (7, 1, CSTR + 1)]
                    pend = []
                    for ci, bcol, slot in ((0, 1, 1), (1, 4, 4), (2, 5, 5)):
                        chv = (v1p[:, CSPAN:2 * CSPAN] if ci == 2 else
                               (v1p[:, 0:CSPAN] if ci == 1 else k1p[:, 0:CSPAN]))
                        m3 = chv[:, 1:1 + HWP].rearrange("p (h w) -> p h w", w=WP)
                        for g in range(8):
                            blk = blkp.tile([P, 3 * RBN], bf16, name="blk")
                            pss = [psD.tile([P, 392], f32, name=f"dw{j}")
                                   for j in range(3)]
                            for pi, (ta, cpi, delta) in enumerate(pairs):
                                dya, dxa = ta // 3, ta % 3
                                lhsT = dgv[:, ci * 9 + ta:ci * 9 + ta + 2, :]
                                for j in range(3):
                                    rb = g * 3 + j
                                    off = (cpi * CSTR + 1
                                           + (4 * rb + dya) * WP + (dxa - 1))
                                    va = chv[:, off:off + 392]
                                    rhs = bass.AP(
                                        tensor=va.tensor, offset=va.offset,
                                        ap=[va.ap[0], [delta, 2], [1, 392]])
                                    nc.tensor.matmul(
                                        pss[j][:], lhsT, rhs,
                                        start=(pi == 0), stop=False, perf_mode=DR)
                            for j in range(3):
                                rb = g * 3 + j
                                dwv = pss[j][:].rearrange(
                                    "p (h w) -> p h w", w=WP)[:, :, 1:97]
                                nc.tensor.matmul(
                                    dwv, dgv[:, ci * 9 + 4, :],
                                    m3[:, 4 * rb + 1:4 * rb + 5, 1:97],
                                    start=False, stop=True)
                                nc.scalar.activation(
                                    blk[:, j * RBN:(j + 1) * RBN].rearrange(
                                        "p (h w) -> p h w", w=96),
                                    dwv, AF.Relu, bias=c_bias[:, bcol:bcol + 1])
                            bv = blk[:].rearrange(
                                "p (r h wq ws) -> p r wq h ws", r=3, h=4, ws=4)
                            nc.vector.tensor_reduce(
                                p24[:, slot * 576 + g * 72:slot * 576 + (g + 1) * 72]
                                .rearrange("p (a b) -> p a b", a=3),
                                bv, axis=AX.XY, op=ALU.add)
                            # spread previous chunk's finish work across groups
                            if pend and g % 2 == 1:
                                pend.pop(0)()
                        while pend:
                            pend.pop(0)()

                        def key_done():
                            smallpools(0, 2)
                            for kq in range(2):
                                nc.vector.tensor_mul(
                                    keyn8[:, kq * 128:kq * 128 + S],
                                    allp[:, kq * S:(kq + 1) * S], c_scl[:, 0:S])
                            do_simT(0); do_exp(0)

                        def b_pre0():
                            do_sumnorm(0); do_simT(1); do_exp(1)
                            do_xbt(0); do_xbt(1)

                        def b_pre1():
                            do_sumnorm(1); do_simT(2); do_exp(2)

                        if ci == 0:
                            pend = [key_done, b_pre0,
                                    lambda: val_pools(2, 4),
                                    lambda: (vt_build(0), vt_build(1))]
                        elif ci == 1:
                            pend = [lambda: val_pools(4, 5), b_pre1,
                                    lambda: vt_build(2)]
                        else:
                            val_pools(5, 6)
                            do_sumnorm(2); do_simT(3); do_exp(3)
                            do_simT(4); do_exp(4)
                            vt_build(3)

            # ---------------- Phase B: context / output ----------------
            with ExitStack() as bctx:
                obp = bctx.enter_context(tc.tile_pool(name="outb", bufs=3))
                psC = bctx.enter_context(tc.tile_pool(name="psC", bufs=4, space="PSUM"))

                for n in range(NCH):
                    if n + 5 < NCH:
                        do_simT(n + 5)
                        do_exp(n + 5)
                    if n + 3 < NCH:
                        do_sumnorm(n + 3)
                    if n + 2 < NCH:
                        do_xbt(n + 2)
                    outb = obp.tile([P, 4 * NCW], bf16, name="outb")
                    xbt = xbts[n][:]
                    for cv in range(4):
                        cps = psC.tile([P, NCW], f32, name="ctx")
                        nc.tensor.matmul(cps[:], vT[:, cv * 128:(cv + 1) * 128],
                                         etns[n][:], start=True,
                                         stop=(cv < 2))
                        if cv < 2:
                            nc.vector.tensor_add(
                                outb[:, cv * NCW:(cv + 1) * NCW], cps[:],
                                xbt[:, cv * NCW:(cv + 1) * NCW])
                        else:
                            nc.tensor.matmul(cps[:], c_id[:],
                                             xbt[:, cv * NCW:(cv + 1) * NCW],
                                             start=False, stop=True)
                            nc.scalar.copy(outb[:, cv * NCW:(cv + 1) * NCW],
                                           cps[:])
                    nc.scalar.dma_start(
                        y_r[:, :, n * NCW:(n + 1) * NCW],
                        outb[:].rearrange("p (t n) -> p t n", t=4))

    nc.compile()
    return nc


def prep_host_inputs(inputs):
    """Fold BN affine into weights, quantize to fp8, build aux tensors."""
    import ml_dtypes
    F8 = ml_dtypes.float8_e4m3
    g = lambda a: np.ascontiguousarray(np.asarray(a, dtype=np.float32))
    wq = (g(inputs["q_g"])[:, None] * g(inputs["q_w"])[:, :, 0, 0]).T
    wkp = (g(inputs["kp_g"])[:, None] * g(inputs["kp_w"])[:, :, 0, 0]).T
    wvp = (g(inputs["vp_g"])[:, None] * g(inputs["vp_w"])[:, :, 0, 0]).T
    wkc = g(inputs["kc_g"])[:, None] * g(inputs["kc_w"])[:, 0].reshape(128, 9)
    wvc = g(inputs["vc_g"])[:, None] * g(inputs["vc_w"])[:, 0].reshape(256, 9)

    diag = np.zeros((3, 9, 128, 128), np.float32)
    for t in range(9):
        diag[0, t] = np.diag(wkc[:, t])
        diag[1, t] = np.diag(wvc[:128, t])
        diag[2, t] = np.diag(wvc[128:, t])

    scale110 = np.zeros(S, np.float32)
    scale110[0] = 1.0 / 9216
    scale110[1:10] = 1.0 / 1024
    scale110[10:46] = 1.0 / 256
    scale110[46:110] = 1.0 / 144
    scl = np.zeros((2, 128, S), np.float32)
    scl[0] = scale110 / 16.0
    scl[1] = scale110

    bias = np.zeros((128, 8), np.float32)
    bias[:, 0] = g(inputs["kp_b"])
    bias[:, 1] = g(inputs["kc_b"])
    bias[:, 2] = g(inputs["vp_b"])[:128]
    bias[:, 3] = g(inputs["vp_b"])[128:]
    bias[:, 4] = g(inputs["vc_b"])[:128]
    bias[:, 5] = g(inputs["vc_b"])[128:]
    bias[:, 6] = g(inputs["q_b"])[:128]
    bias[:, 7] = g(inputs["q_b"])[128:]

    return {
        "wq": np.ascontiguousarray(wq).astype(F8),
        "wkp": np.ascontiguousarray(wkp).astype(F8),
        "wvp": np.ascontiguousarray(wvp).astype(F8),
        "diag": diag.astype(F8),
        "ident": np.eye(128, dtype=ml_dtypes.bfloat16),
        "ones": np.ones((128, 128), dtype=ml_dtypes.bfloat16),
        "scl": scl,
        "bias": bias,
    }


def make_in_maps(inputs):
    import ml_dtypes
    host = prep_host_inputs(inputs)
    x = np.asarray(inputs["x"], dtype=np.float32)
    B = x.shape[0]
    in_maps = []
    for b in range(B):
        m = dict(host)
        xf = np.ascontiguousarray(x[b].reshape(512, HW))
        m["x8"] = xf.astype(ml_dtypes.float8_e4m3)
        m["xb"] = xf.astype(ml_dtypes.bfloat16)
        in_maps.append(m)
    return in_maps


_NC = None


def get_nc():
    global _NC
    if _NC is None:
        _NC = build_bass()
    return _NC


def kernel(**inputs):
    from concourse import bass_utils
    nc = get_nc()
    in_maps = make_in_maps(inputs)
    res = bass_utils.run_bass_kernel_spmd(
        nc, in_maps, core_ids=list(range(len(in_maps))), trace=False)
    outs = [r["y"].astype(np.float32).reshape(512, HH, HH) for r in res.results]
    return np.stack(outs, axis=0).astype(np.float32)


# revision 37
# speedup vs baseline: 1.2233x; 1.0578x over previous
"""CAPAttentionModule Trainium2 kernel (v3: fp8 DoubleRow + phase overlap).

Data-parallel over batch: 8 images -> 8 NeuronCores, one image per core.
Per core (x: [512, 9216] = [C, H*W], H=W=96):
  k1 = relu(Wkp x + b)   [128, HW]  fp8 DoubleRow conv (K=512, 2 instrs/tile)
  v1 = relu(Wvp x + b)   [256, HW]  fp8 DoubleRow conv
  q  = relu(Wq x + b)    [256, HW]  fp8 DoubleRow conv, kept fp8 for sim
  k2/v2 = relu(dw3x3+b)  diag matmuls: 4 fp8 DoubleRow tap-pairs + 1 single.
         Maps are stored 3x at byte offsets {0, +9615, +19230}; 9615 % 16 ==
         15 makes every cross-copy kt stride 16B-aligned, so ALL tap pairs
         satisfy the fp8 dual restrictions. Copies ride the gpsimd DMA ring.
  key/value = psp pooling  [*, 110]  strip reduces on DVE inside the dw window
  simT = key^T q / 16      [110, HW] ONE fp8 DoubleRow matmul per 512-chunk
  softmax over s (= partitions): exp on ACT; sum via ones-matmul broadcast;
         reciprocal_approx_fast + scale on DVE
  out = x + value @ simT   residual: half DVE adds, half identity-matmul
         PSUM accumulation + ACT copy
Phase B's sim/softmax pipeline is issued up to 5 chunks deep starting inside
the depthwise phase, so the PE never idles across the A/B boundary.
HBM traffic: read x fp8 (4.7MB) + x bf16 (9.4MB), write y bf16 (9.4MB).
"""

import numpy as np

P = 128
HH = 96
WP = 98          # padded width (zero border ring)
HW = 9216
HWP = WP * WP    # 9604
RB = 24          # row blocks of 4 rows
RBN = 4 * HH     # 384
NCH = 18         # phase-B column chunks
NCW = 512
S = 110
CSTR = 9615      # map copy stride (== 15 mod 16 -> aligned fp8 DR kt strides)
CSPAN = 1 + 2 * CSTR + HWP + 10   # one map chunk: lead pad + 3 copies


def build_bass():
    import concourse.bacc as bacc
    import concourse.tile as tile
    from concourse import mybir, bass
    from contextlib import ExitStack

    f32 = mybir.dt.float32
    f8 = mybir.dt.float8e4
    bf16 = mybir.dt.bfloat16
    AF = mybir.ActivationFunctionType
    AX = mybir.AxisListType
    ALU = mybir.AluOpType
    DR = mybir.MatmulPerfMode.DoubleRow

    nc = bacc.Bacc("TRN2", target_bir_lowering=False, debug=False,
                   enable_asserts=False, num_devices=8)

    x8_d = nc.dram_tensor("x8", [512, HW], f8, kind="ExternalInput").ap()
    xb_d = nc.dram_tensor("xb", [512, HW], bf16, kind="ExternalInput").ap()
    wq_d = nc.dram_tensor("wq", [512, 256], f8, kind="ExternalInput").ap()
    wkp_d = nc.dram_tensor("wkp", [512, 128], f8, kind="ExternalInput").ap()
    wvp_d = nc.dram_tensor("wvp", [512, 256], f8, kind="ExternalInput").ap()
    diag_d = nc.dram_tensor("diag", [3, 9, 128, 128], f8, kind="ExternalInput").ap()
    id_d = nc.dram_tensor("ident", [128, 128], bf16, kind="ExternalInput").ap()
    ones_d = nc.dram_tensor("ones", [128, 128], bf16, kind="ExternalInput").ap()
    scl_d = nc.dram_tensor("scl", [2, 128, S], f32, kind="ExternalInput").ap()
    bias_d = nc.dram_tensor("bias", [128, 8], f32, kind="ExternalInput").ap()
    y_d = nc.dram_tensor("y", [512, HW], bf16, kind="ExternalOutput").ap()

    x8_r = x8_d.rearrange("(t p) n -> p t n", p=P)
    xb_r = xb_d.rearrange("(t p) n -> p t n", p=P)
    y_r = y_d.rearrange("(t p) n -> p t n", p=P)

    with tile.TileContext(nc) as tc:
        with ExitStack() as top:
            cpool = top.enter_context(tc.tile_pool(name="consts", bufs=1))
            kpool = top.enter_context(tc.tile_pool(name="keep", bufs=1))
            # pools that span the A/B boundary
            pp = top.enter_context(tc.tile_pool(name="pexp", bufs=5))
            sp = top.enter_context(tc.tile_pool(name="small", bufs=3))
            xbp = top.enter_context(tc.tile_pool(name="xbt", bufs=4))

            c_wkp = cpool.tile([P, 4 * 128], f8)
            nc.scalar.dma_start(c_wkp[:].rearrange("p (t m) -> p t m", t=4),
                                wkp_d.rearrange("(t p) m -> p t m", p=P))
            c_wvp = cpool.tile([P, 4 * 256], f8)
            nc.scalar.dma_start(c_wvp[:].rearrange("p (t m) -> p t m", t=4),
                                wvp_d.rearrange("(t p) m -> p t m", p=P))
            c_wq = cpool.tile([P, 4 * 256], f8)
            nc.scalar.dma_start(c_wq[:].rearrange("p (t m) -> p t m", t=4),
                                wq_d.rearrange("(t p) m -> p t m", p=P))
            c_bias = cpool.tile([P, 8], f32)
            nc.scalar.dma_start(c_bias[:], bias_d)
            # deferred consts (first needed at dw / phase B) on scalar ring
            c_dg = cpool.tile([P, 27 * 128], f8)
            nc.scalar.dma_start(c_dg[:].rearrange("p (ct m) -> p ct m", ct=27),
                                diag_d.rearrange("c t p m -> p (c t) m"))
            c_id = cpool.tile([P, 128], bf16)
            nc.scalar.dma_start(c_id[:], id_d)
            c_ones = cpool.tile([P, 128], bf16)
            nc.scalar.dma_start(c_ones[:], ones_d)
            c_scl = cpool.tile([P, 2 * S], f32)
            nc.scalar.dma_start(c_scl[:].rearrange("p (s m) -> p s m", s=2),
                                scl_d.rearrange("s p m -> p s m"))

            keyn8 = kpool.tile([P, 2 * 128], f8)      # [kt, 110(+pad)] key/16
            vT = kpool.tile([S, 512], bf16)           # value^T [s, c]
            qsb8 = kpool.tile([P, 2 * HW], f8)        # q in fp8 [kq, n]

            # ---- phase-B helper state (pipeline spans into phase A) ----
            sims = [None] * NCH
            ets = [None] * NCH
            etns = [None] * NCH
            xbts = [None] * NCH
            bfn = {}

            def do_simT(n):
                knv = keyn8[:].rearrange("p (t m) -> p t m", t=2)[:, :, 0:S]
                qv = qsb8[:].rearrange("p (t m) -> p t m", t=2)
                ps = bfn["psS"].tile([S, NCW], f32, name="sim")
                nc.tensor.matmul(ps[:], knv, qv[:, :, n * NCW:(n + 1) * NCW],
                                 start=True, stop=True, perf_mode=DR)
                sims[n] = ps

            def do_exp(n):
                eT = pp.tile([S, NCW], bf16, name="eT")
                nc.scalar.activation(eT[:], sims[n][:], AF.Exp)
                ets[n] = eT

            def do_sumnorm(n):
                sb = bfn["psB"].tile([P, NCW], f32, name="sumbc")
                nc.tensor.matmul(sb[:], c_ones[0:S, :], ets[n][:],
                                 start=True, stop=True)
                rbc = sp.tile([S, NCW], f32, name="rbc")
                nc.vector.reciprocal_approx_fast(rbc[:], sb[0:S, :])
                eTn = pp.tile([S, NCW], bf16, name="eTn")
                nc.gpsimd.tensor_mul(eTn[:], ets[n][:], rbc[:])
                etns[n] = eTn

            def do_xbt(n):
                xbt = xbp.tile([P, 4 * NCW], bf16, name="xtb")
                nc.sync.dma_start(
                    xbt[:].rearrange("p (t n) -> p t n", t=4),
                    xb_r[:, :, n * NCW:(n + 1) * NCW])
                xbts[n] = xbt

            # ---------------- Phase A ----------------
            with ExitStack() as actx:
                bigp = actx.enter_context(tc.tile_pool(name="bigA", bufs=1))
                xap = actx.enter_context(tc.tile_pool(name="xa", bufs=3))
                blkp = actx.enter_context(tc.tile_pool(name="blk", bufs=3))
                tmpp = actx.enter_context(tc.tile_pool(name="tmpA", bufs=1))

                k1p = bigp.tile([P, CSPAN], f8)
                v1p = bigp.tile([P, 2 * CSPAN], f8)
                p24 = bigp.tile([P, 6 * 576], f32)
                allp = bigp.tile([P, 6 * S], f32)
                valn = bigp.tile([P, 4 * S], bf16)

                # zero pad borders (rows 0/97, cols 0/97 of each padded map)
                for chv in (k1p[:, 1:1 + HWP], v1p[:, 1:1 + HWP],
                            v1p[:, CSPAN + 1:CSPAN + 1 + HWP]):
                    c3 = chv.rearrange("p (h w) -> p h w", w=WP)
                    nc.gpsimd.memset(c3[:, 0:1, :], 0.0)
                    nc.gpsimd.memset(c3[:, 97:98, :], 0.0)
                    nc.gpsimd.memset(c3[:, 1:97, 0:1], 0.0)
                    nc.gpsimd.memset(c3[:, 1:97, 97:98], 0.0)

                def strip_pool(mapap, base, slot, strip):
                    # 4x4 block sums of 24 rows (6 rowblocks) -> p24
                    off = base + (24 * strip + 1) * WP + 1
                    src = bass.AP(tensor=mapap.tensor, offset=mapap.offset + off,
                                  ap=[mapap.ap[0], [4 * WP, 6], [4, 24],
                                      [WP, 4], [1, 4]])
                    dst = p24[:, slot * 576 + strip * 144:slot * 576 + (strip + 1) * 144]
                    nc.vector.tensor_reduce(
                        dst.rearrange("p (a b) -> p a b", a=6), src,
                        axis=AX.XY, op=ALU.add)

                def strip_copy(mapap, cb, strip):
                    # replicate newly-written strip rows into copies 2 and 3
                    r0 = [0, 2450, 4802, 7154][strip]
                    r1 = [2450, 4802, 7154, 9604][strip]
                    src = mapap[:, cb + 1 + r0:cb + 1 + r1]
                    for cc in (1, 2):
                        nc.gpsimd.dma_start(
                            mapap[:, cb + 1 + cc * CSTR + r0:cb + 1 + cc * CSTR + r1],
                            src)

                # primary 1x1 convs + q conv, fp8 DoubleRow, 2 rowblocks/DMA
                with tc.tile_pool(name="psA", bufs=2, space="PSUM") as psA, \
                        tc.tile_pool(name="psQ", bufs=1, space="PSUM") as psQ:
                    for rbb in range(RB // 2):
                        xt = xap.tile([P, 4 * 2 * RBN], f8, name="xt")
                        nc.sync.dma_start(
                            xt[:].rearrange("p (t n) -> p t n", t=4),
                            x8_r[:, :, rbb * 2 * RBN:(rbb + 1) * 2 * RBN])
                        xtv = xt[:].rearrange("p (t n) -> p t n", t=4)
                        wqv = c_wq[:].rearrange("p (t m) -> p t m", t=4)
                        wkv = c_wkp[:].rearrange("p (t m) -> p t m", t=4)
                        wvv = c_wvp[:].rearrange("p (t m) -> p t m", t=4)
                        for sub in range(2):
                            rb = rbb * 2 + sub
                            rhs0 = xtv[:, 0:2, sub * RBN:(sub + 1) * RBN]
                            rhs1 = xtv[:, 2:4, sub * RBN:(sub + 1) * RBN]
                            dsts = [(k1p, 0, wkv[:, :, 0:128], 0),
                                    (v1p, 0, wvv[:, :, 0:128], 2),
                                    (v1p, 1, wvv[:, :, 128:256], 3)]
                            for di, (dst, half, wt, bcol) in enumerate(dsts):
                                ps = psA.tile([P, RBN], f32, name=f"pps{di}")
                                nc.tensor.matmul(ps[:], wt[:, 0:2, :], rhs0,
                                                 start=True, stop=False, perf_mode=DR)
                                nc.tensor.matmul(ps[:], wt[:, 2:4, :], rhs1,
                                                 start=False, stop=True, perf_mode=DR)
                                dv = dst[:, half * CSPAN + 1:half * CSPAN + 1 + HWP].rearrange(
                                    "p (h w) -> p h w", w=WP)
                                dvv = dv[:, 4 * rb + 1:4 * rb + 5, 1:97]
                                psv = ps[:].rearrange("p (h w) -> p h w", w=HH)
                                if di == 2:
                                    nc.vector.tensor_scalar(
                                        dvv, psv, c_bias[:, bcol:bcol + 1],
                                        0.0, ALU.add, ALU.max)
                                else:
                                    nc.scalar.activation(
                                        dvv, psv, AF.Relu,
                                        bias=c_bias[:, bcol:bcol + 1])
                            for kq in range(2):
                                qps = psQ.tile([P, RBN], f32, name=f"q{kq}")
                                nc.tensor.matmul(qps[:], wqv[:, 0:2, kq * 128:kq * 128 + 128],
                                                 rhs0, start=True, stop=False, perf_mode=DR)
                                nc.tensor.matmul(qps[:], wqv[:, 2:4, kq * 128:kq * 128 + 128],
                                                 rhs1, start=False, stop=True, perf_mode=DR)
                                qdst = qsb8[:, kq * HW + rb * RBN:kq * HW + (rb + 1) * RBN]
                                if kq == 0:
                                    nc.vector.tensor_scalar(
                                        qdst, qps[:], c_bias[:, 6:7], 0.0,
                                        ALU.add, ALU.max)
                                else:
                                    nc.scalar.activation(
                                        qdst, qps[:], AF.Relu,
                                        bias=c_bias[:, 7:8])
                        if rbb % 3 == 2:
                            strip = rbb // 3
                            strip_copy(k1p[:], 0, strip)
                            strip_copy(v1p[:], 0, strip)
                            strip_copy(v1p[:], CSPAN, strip)
                            strip_pool(k1p[:], 1, 0, strip)
                            strip_pool(v1p[:], 1, 2, strip)
                            strip_pool(v1p[:], CSPAN + 1, 3, strip)

                # phase-B psum pools: allocated now (primary pools released)
                # on the top stack so they survive into the ctx loop
                bfn["psS"] = top.enter_context(
                    tc.tile_pool(name="psS", bufs=2, space="PSUM"))
                bfn["psB"] = top.enter_context(
                    tc.tile_pool(name="psB", bufs=1, space="PSUM"))

                # small pools over map range [m0, m1) -> allp columns
                def smallpools(m0, m1):
                    m = m1 - m0
                    allp_v = allp[:, m0 * S:m1 * S].rearrange(
                        "p (m s) -> p m s", s=S)
                    p24s = p24[:, m0 * 576:m1 * 576]
                    nc.vector.reduce_sum(
                        allp_v[:, :, 0:1],
                        p24s.rearrange("p (m s) -> p m s", s=576), axis=AX.X)
                    tmp = tmpp.tile([P, 1152], f32, name="tmp", tag="tmp")
                    nc.vector.reduce_sum(
                        tmp[:, 0:m * 72],
                        p24s.rearrange("p (mh wq ws) -> p mh wq ws", wq=3, ws=8),
                        axis=AX.X)
                    nc.vector.reduce_sum(
                        allp_v[:, :, 1:10],
                        tmp[:, 0:m * 72].rearrange(
                            "p (m hq hs wq) -> p m hq wq hs", m=m, hq=3, hs=8),
                        axis=AX.X)
                    tmp6 = tmpp.tile([P, 1152], f32, name="tmp6", tag="tmp")
                    nc.vector.reduce_sum(
                        tmp6[:, 0:m * 144],
                        p24s.rearrange("p (mh wq ws) -> p mh wq ws", wq=6, ws=4),
                        axis=AX.X)
                    nc.vector.reduce_sum(
                        allp_v[:, :, 10:46],
                        tmp6[:, 0:m * 144].rearrange(
                            "p (m hq hs wq) -> p m hq wq hs", m=m, hq=6, hs=4),
                        axis=AX.X)
                    tmp8 = tmpp.tile([P, 1152], f32, name="tmp8", tag="tmp")
                    nc.vector.reduce_sum(
                        tmp8[:, 0:m * 192],
                        p24s.rearrange("p (mh wq ws) -> p mh wq ws", wq=8, ws=3),
                        axis=AX.X)
                    nc.vector.reduce_sum(
                        allp_v[:, :, 46:110],
                        tmp8[:, 0:m * 192].rearrange(
                            "p (m hq hs wq) -> p m hq wq hs", m=m, hq=8, hs=3),
                        axis=AX.X)

                # depthwise 3x3: 4 fp8-DR tap pairs + 1 single per rowblock
                with tc.tile_pool(name="psD", bufs=1, space="PSUM") as psD, \
                        tc.tile_pool(name="psTa", bufs=2, space="PSUM") as psTp:

                    def vt_build(j):
                        tp = psTp.tile([P, 128], bf16, name="tp", tag="tp")
                        nc.tensor.transpose(tp[0:S, :], valn[:, j * S:(j + 1) * S],
                                            c_id[:])
                        nc.scalar.copy(vT[:, j * 128:(j + 1) * 128], tp[0:S, :])

                    def val_pools(m0, m1):
                        smallpools(m0, m1)
                        for mm in range(m0, m1):
                            j = mm - 2
                            nc.vector.tensor_mul(valn[:, j * S:(j + 1) * S],
                                                 allp[:, mm * S:(mm + 1) * S],
                                                 c_scl[:, S:2 * S])

                    def val_finish(m0, m1):
                        val_pools(m0, m1)
                        for mm in range(m0, m1):
                            vt_build(mm - 2)

                    dgv = c_dg[:].rearrange("p (ct m) -> p ct m", ct=27)
                    # tap pairs for fp8 DoubleRow: (ta, kt0 copy idx, kt delta)
                    pairs = [(0, 0, CSTR + 1), (2, 0, 96),
                             (5, 0, 96), (7, 1, CSTR + 1)]
                    pend = []
                    for ci, bcol, slot in ((0, 1, 1), (1, 4, 4), (2, 5, 5)):
                        chv = (v1p[:, CSPAN:2 * CSPAN] if ci == 2 else
                               (v1p[:, 0:CSPAN] if ci == 1 else k1p[:, 0:CSPAN]))
                        m3 = chv[:, 1:1 + HWP].rearrange("p (h w) -> p h w", w=WP)
                        for g in range(8):
                            blk = blkp.tile([P, 3 * RBN], bf16, name="blk")
                            pss = [psD.tile([P, 392], f32, name=f"dw{j}")
                                   for j in range(3)]
                            for pi, (ta, cpi, delta) in enumerate(pairs):
                                dya, dxa = ta // 3, ta % 3
                                lhsT = dgv[:, ci * 9 + ta:ci * 9 + ta + 2, :]
                                for j in range(3):
                                    rb = g * 3 + j
                                    off = (cpi * CSTR + 1
                                           + (4 * rb + dya) * WP + (dxa - 1))
                                    va = chv[:, off:off + 392]
                                    rhs = bass.AP(
                                        tensor=va.tensor, offset=va.offset,
                                        ap=[va.ap[0], [delta, 2], [1, 392]])
                                    nc.tensor.matmul(
                                        pss[j][:], lhsT, rhs,
                                        start=(pi == 0), stop=False, perf_mode=DR)
                            for j in range(3):
                                rb = g * 3 + j
                                dwv = pss[j][:].rearrange(
                                    "p (h w) -> p h w", w=WP)[:, :, 1:97]
                                nc.tensor.matmul(
                                    dwv, dgv[:, ci * 9 + 4, :],
                                    m3[:, 4 * rb + 1:4 * rb + 5, 1:97],
                                    start=False, stop=True)
                                nc.scalar.activation(
                                    blk[:, j * RBN:(j + 1) * RBN].rearrange(
                                        "p (h w) -> p h w", w=96),
                                    dwv, AF.Relu, bias=c_bias[:, bcol:bcol + 1])
                            bv = blk[:].rearrange(
                                "p (r h wq ws) -> p r wq h ws", r=3, h=4, ws=4)
                            nc.vector.tensor_reduce(
                                p24[:, slot * 576 + g * 72:slot * 576 + (g + 1) * 72]
                                .rearrange("p (a b) -> p a b", a=3),
                                bv, axis=AX.XY, op=ALU.add)
                            # spread previous chunk's finish work across groups
                            if pend and g % 2 == 1:
                                pend.pop(0)()
                        while pend:
                            pend.pop(0)()

                        def key_done():
                            smallpools(0, 2)
                            for kq in range(2):
                                nc.vector.tensor_mul(
                                    keyn8[:, kq * 128:kq * 128 + S],
                                    allp[:, kq * S:(kq + 1) * S], c_scl[:, 0:S])
                            do_simT(0); do_exp(0)

                        def b_pre0():
                            do_sumnorm(0); do_simT(1); do_exp(1)
                            do_xbt(0); do_xbt(1)

                        def b_pre1():
                            do_sumnorm(1); do_simT(2); do_exp(2)

                        if ci == 0:
                            pend = [key_done, b_pre0,
                                    lambda: val_pools(2, 4),
                                    lambda: (vt_build(0), vt_build(1))]
                        elif ci == 1:
                            pend = [lambda: val_pools(4, 5), b_pre1,
                                    lambda: vt_build(2)]
                        else:
                            val_pools(5, 6)
                            do_sumnorm(2); do_simT(3); do_exp(3)
                            do_simT(4); do_exp(4)
                            vt_build(3)

            # ---------------- Phase B: context / output ----------------
            with ExitStack() as bctx:
                obp = bctx.enter_context(tc.tile_pool(name="outb", bufs=3))
                psC = bctx.enter_context(tc.tile_pool(name="psC", bufs=4, space="PSUM"))

                for n in range(NCH):
                    if n + 5 < NCH:
                        do_simT(n + 5)
                        do_exp(n + 5)
                    if n + 3 < NCH:
                        do_sumnorm(n + 3)
                    if n + 2 < NCH:
                        do_xbt(n + 2)
                    outb = obp.tile([P, 4 * NCW], bf16, name="outb")
                    xbt = xbts[n][:]
                    for cv in range(4):
                        cps = psC.tile([P, NCW], f32, name="ctx")
                        nc.tensor.matmul(cps[:], vT[:, cv * 128:(cv + 1) * 128],
                                         etns[n][:], start=True,
                                         stop=(cv < 2))
                        if cv < 2:
                            nc.vector.tensor_add(
                                outb[:, cv * NCW:(cv + 1) * NCW], cps[:],
                                xbt[:, cv * NCW:(cv + 1) * NCW])
                        else:
                            nc.tensor.matmul(cps[:], c_id[:],
                                             xbt[:, cv * NCW:(cv + 1) * NCW],
                                             start=False, stop=True)
                            nc.scalar.copy(outb[:, cv * NCW:(cv + 1) * NCW],
                                           cps[:])
                    nc.scalar.dma_start(
                        y_r[:, :, n * NCW:(n + 1) * NCW],
                        outb[:].rearrange("p (t n) -> p t n", t=4))

    nc.compile()
    return nc


def prep_host_inputs(inputs):
    """Fold BN affine into weights, quantize to fp8, build aux tensors."""
    import ml_dtypes
    F8 = ml_dtypes.float8_e4m3
    g = lambda a: np.ascontiguousarray(np.asarray(a, dtype=np.float32))
    wq = (g(inputs["q_g"])[:, None] * g(inputs["q_w"])[:, :, 0, 0]).T
    wkp = (g(inputs["kp_g"])[:, None] * g(inputs["kp_w"])[:, :, 0, 0]).T
    wvp = (g(inputs["vp_g"])[:, None] * g(inputs["vp_w"])[:, :, 0, 0]).T
    wkc = g(inputs["kc_g"])[:, None] * g(inputs["kc_w"])[:, 0].reshape(128, 9)
    wvc = g(inputs["vc_g"])[:, None] * g(inputs["vc_w"])[:, 0].reshape(256, 9)

    diag = np.zeros((3, 9, 128, 128), np.float32)
    for t in range(9):
        diag[0, t] = np.diag(wkc[:, t])
        diag[1, t] = np.diag(wvc[:128, t])
        diag[2, t] = np.diag(wvc[128:, t])

    scale110 = np.zeros(S, np.float32)
    scale110[0] = 1.0 / 9216
    scale110[1:10] = 1.0 / 1024
    scale110[10:46] = 1.0 / 256
    scale110[46:110] = 1.0 / 144
    scl = np.zeros((2, 128, S), np.float32)
    scl[0] = scale110 / 16.0
    scl[1] = scale110

    bias = np.zeros((128, 8), np.float32)
    bias[:, 0] = g(inputs["kp_b"])
    bias[:, 1] = g(inputs["kc_b"])
    bias[:, 2] = g(inputs["vp_b"])[:128]
    bias[:, 3] = g(inputs["vp_b"])[128:]
    bias[:, 4] = g(inputs["vc_b"])[:128]
    bias[:, 5] = g(inputs["vc_b"])[128:]
    bias[:, 6] = g(inputs["q_b"])[:128]
    bias[:, 7] = g(inputs["q_b"])[128:]

    return {
        "wq": np.ascontiguousarray(wq).astype(F8),
        "wkp": np.ascontiguousarray(wkp).astype(F8),
        "wvp": np.ascontiguousarray(wvp).astype(F8),
        "diag": diag.astype(F8),
        "ident": np.eye(128, dtype=ml_dtypes.bfloat16),
        "ones": np.ones((128, 128), dtype=ml_dtypes.bfloat16),
        "scl": scl,
        "bias": bias,
    }


def make_in_maps(inputs):
    import ml_dtypes
    host = prep_host_inputs(inputs)
    x = np.asarray(inputs["x"], dtype=np.float32)
    B = x.shape[0]
    in_maps = []
    for b in range(B):
        m = dict(host)
        xf = np.ascontiguousarray(x[b].reshape(512, HW))
        m["x8"] = xf.astype(ml_dtypes.float8_e4m3)
        m["xb"] = xf.astype(ml_dtypes.bfloat16)
        in_maps.append(m)
    return in_maps


_NC = None


def get_nc():
    global _NC
    if _NC is None:
        _NC = build_bass()
    return _NC


def kernel(**inputs):
    from concourse import bass_utils
    nc = get_nc()
    in_maps = make_in_maps(inputs)
    res = bass_utils.run_bass_kernel_spmd(
        nc, in_maps, core_ids=list(range(len(in_maps))), trace=False)
    outs = [r["y"].astype(np.float32).reshape(512, HH, HH) for r in res.results]
    return np.stack(outs, axis=0).astype(np.float32)
